# revision 32
# baseline (speedup 1.0000x reference)
"""Causal self-attention on 8 TRN2 NeuronCores (Bass/Tile, SPMD), head-split.

Problem: B=4, T=2048, C=1024, H=16, D=64, fp32 in/out.

Sharding: core i = (batch b=i//2, head-group hg=i%2). Each core computes its
8 heads (4 head-pairs j) for ALL 2048 queries of its batch, with TRUE causal
extents (chunk c of 256 queries attends to exactly 2c+2 key-tiles of 128).
Every core runs the identical instruction stream (SPMD). The output
projection produces a partial y[2048, 1024] (contraction over the core's
512 C-rows); the host sums the two partials per batch and adds the bias.

Schedule: QKV projection for token-slab s+1 and the output projection run
as fine-grained PE filler steps interleaved into the attention chunk loop
(which is ACT/exp-heavy). Slab-0 waves allocate PSUM from the same ss ring
as attention (no pool barrier); k-w1/v-w1 of slab 0 are deferred into
chunk 0. Chunks 6 and 7 are interleaved j-wise so their exp load spreads
and the final normalize->projection tail is short. Mask is a 0/1
multiplicative bf16 applied on VectorE after exp. PE warmup matmuls run
during the initial DMA wait so HAM is at K=8/8 when real work arrives.
"""
import os
import numpy as np
import ml_dtypes

import concourse.bacc as bacc
import concourse.mybir as mybir
import concourse.tile as tile
from concourse.bass_utils import run_bass_kernel_spmd

B, T, C, H, D = 4, 2048, 1024, 16, 64
QC = 256                      # q-chunk width
NCH = 8                       # q-chunks per core (all of T)
NJ = 4                        # head-pairs per core
CH = 512                      # C-half per core
F32 = mybir.dt.float32
BF16 = mybir.dt.bfloat16
VA_W = 8 * (D + 1)            # 520: V_aug cols = 8 heads x (64 | ones)

_cache = {}


def _build():
    nc = bacc.Bacc("TRN2", target_bir_lowering=False, debug=False,
                   enable_asserts=False, num_devices=8)

    def din(name, shape, dt=BF16):
        return nc.dram_tensor(name, list(shape), dt, kind="ExternalInput").ap()

    xt_d = din("xt", (C, T))            # x[b].T
    wq_d = din("wq", (C, CH))           # head-half cols, pre-scaled by 1/8
    wk_d = din("wk", (C, CH))
    wv_d = din("wv", (C, CH))
    wp_d = din("wp", (CH, C))           # head-half rows
    bq_d = din("bq", (4, 128, 1), F32)  # pre-scaled by 1/8
    bk_d = din("bk", (4, 128, 1), F32)
    mk_d = din("masks", (NCH, 128, 4 * QC))  # multiplicative 0/1
    y_d = nc.dram_tensor("y", [T, C], BF16, kind="ExternalOutput").ap()

    bypass = mybir.AluOpType.bypass
    mult = mybir.AluOpType.mult
    EXP = mybir.ActivationFunctionType.Exp

    with tile.TileContext(nc) as tc:
        # ---- PE warmup: keep HAM busy during the initial DMA wait ----
        with tc.tile_pool(name="wu", bufs=1) as wup, \
             tc.tile_pool(name="wups", bufs=1, space="PSUM") as wups:
            wt = wup.tile([128, 512], BF16, name="wt", tag="wt")
            nc.vector.memset(wt[:, 0:8], 0.0)
            wm = [wups.tile([128, 512], F32, name=f"wm{i}", tag=f"wm{i}")
                  for i in range(2)]
            for i in range(14):
                nc.tensor.matmul(out=wm[i % 2][:], lhsT=wt[:, 0:128], rhs=wt[:],
                                 start=True, stop=True, skip_group_check=True)

        # ---------------- persistent tiles ------------------------------
        pers = tc.alloc_tile_pool(name="pers", bufs=1)
        KT = [[pers.tile([128, 512], BF16, name=f"kt{j}_{sl}", tag=f"kt{j}_{sl}")
               for sl in range(4)] for j in range(NJ)]
        QT = [[pers.tile([128, 512], BF16, name=f"qt{j}_{sl}", tag=f"qt{j}_{sl}")
               for sl in range(4)] for j in range(NJ)]
        VA = [pers.tile([128, VA_W], BF16, name=f"va{g}", tag=f"va{g}")
              for g in range(16)]
        OT = [[pers.tile([128, QC], BF16, name=f"ot{j}_{cc}", tag=f"ot{j}_{cc}")
               for cc in range(NCH)] for j in range(NJ)]
        mk_all = pers.tile([128, NCH * 4 * QC], BF16, name="mk_all", tag="mk_all")
        ones8 = pers.tile([128, 8], BF16, name="ones8", tag="ones8")
        nc.vector.memset(ones8[:], 1.0)
        ones8_3d = ones8[:].unsqueeze(2)
        for g in range(16):
            dst1 = VA[g][:].rearrange("p (h d) -> p h d", d=D + 1)[:, :, D:D + 1]
            nc.vector.tensor_copy(out=dst1, in_=ones8_3d)

        with tc.tile_pool(name="sb", bufs=1) as sb, \
             tc.tile_pool(name="psp", bufs=1, space="PSUM") as psp:
            wts = xtp = msc = sb
            kvps = aps = psp
            wq_all = wts.tile([128, 8 * CH], BF16, name="wq_all", tag="wq_all")
            wk_all = wts.tile([128, 8 * CH], BF16, name="wk_all", tag="wk_all")
            wv_all = wts.tile([128, 8 * CH], BF16, name="wv_all", tag="wv_all")
            wp_all = wts.tile([128, 4 * C], BF16, name="wp_all", tag="wp_all")
            bq_all = wts.tile([128, 4], F32, name="bq_all", tag="bq_all")
            bk_all = wts.tile([128, 4], F32, name="bk_all", tag="bk_all")

            def load_xt(slab):
                xt_all = xtp.tile([128, 8 * 512], BF16, name="xt_all", tag="xt_all", bufs=2)
                src = xt_d[:, 512*slab:512*(slab+1)]
                nc.sync.dma_start(
                    out=xt_all[:].rearrange("p (c n) -> p c n", n=512),
                    in_=src.rearrange("(c p) n -> p c n", p=128))
                return [xt_all[:, 512*c:512*(c+1)] for c in range(8)]

            # startup DMAs, interleaved so slab-0 Q can begin after ~256 KB
            xt0_tile = xtp.tile([128, 8 * 512], BF16, name="xt_all", tag="xt_all", bufs=2)
            for c in range(8):
                nc.sync.dma_start(out=wq_all[:, CH*c:CH*(c+1)],
                                  in_=wq_d[128*c:128*(c+1), :])
                nc.sync.dma_start(out=xt0_tile[:, 512*c:512*(c+1)],
                                  in_=xt_d[128*c:128*(c+1), 0:512])
                if c == 0:
                    nc.sync.dma_start(out=bq_all[:].rearrange("p (c one) -> p c one", one=1),
                                      in_=bq_d.rearrange("c p one -> p c one"))
                    nc.sync.dma_start(out=bk_all[:].rearrange("p (c one) -> p c one", one=1),
                                      in_=bk_d.rearrange("c p one -> p c one"))
            xts0 = [xt0_tile[:, 512*c:512*(c+1)] for c in range(8)]
            nc.sync.dma_start(out=wk_all[:].rearrange("p (c n) -> p c n", n=CH),
                              in_=wk_d.rearrange("(c p) n -> p c n", p=128))
            nc.sync.dma_start(out=wv_all[:].rearrange("p (c n) -> p c n", n=CH),
                              in_=wv_d.rearrange("(c p) n -> p c n", p=128))
            nc.sync.dma_start(out=mk_all[:].rearrange("p (s n) -> p s n", s=NCH),
                              in_=mk_d.rearrange("s p n -> p s n"))
            nc.sync.dma_start(out=wp_all[:].rearrange("p (c n) -> p c n", n=C),
                              in_=wp_d.rearrange("(c p) n -> p c n", p=128))

            # ---- filler step machinery ----
            def kv_cell():
                return [kvps.tile([128, 512], F32, name=f"kv{i}", tag=f"kv{i}")
                        for i in range(2)]

            def ss_cell():
                big = aps.tile([128, 1024], F32, name="ss", tag="ss", bufs=2)
                return [big[:, 0:512], big[:, 512:1024]]

            def slab_steps(xts, slab, ss_waves=()):
                """QKV projection for one 512-token slab as fine-grained steps.

                6 waves x 9 steps, order: q-w0, q-w1, k-w0, k-w1, v-w0, v-w1.
                Waves in ss_waves allocate their PSUM from the attention ss
                ring (used by the dense slab-0 pass to avoid 2-bank WAR
                stalls and pool barriers).
                """
                steps = []
                widx = 0
                for kind in ("q", "k"):
                    w_all = wq_all if kind == "q" else wk_all
                    b_all = bq_all if kind == "q" else bk_all
                    dst = QT if kind == "q" else KT
                    for w in range(2):
                        use_ss = widx in ss_waves
                        widx += 1
                        cell = {}
                        def alloc(use_ss=use_ss, cell=cell):
                            cell["p"] = ss_cell() if use_ss else kv_cell()
                        def mstep(c, w=w, w_all=w_all, cell=cell):
                            for i in range(2):
                                j = 2 * w + i
                                nc.tensor.matmul(out=cell["p"][i][:],
                                                 lhsT=w_all[:, CH*c+128*j:CH*c+128*(j+1)],
                                                 rhs=xts[c], start=(c == 0), stop=(c == 7))
                        def evac(w=w, b_all=b_all, dst=dst, cell=cell):
                            for i in range(2):
                                j = 2 * w + i
                                nc.vector.tensor_scalar_add(out=dst[j][slab][:],
                                                            in0=cell["p"][i][:],
                                                            scalar1=b_all[:, j:j+1])
                        for c in range(8):
                            if c == 0:
                                steps.append(lambda c=c, a=alloc, k=mstep: (a(), k(c)))
                            else:
                                steps.append(lambda c=c, k=mstep: k(c))
                        steps.append(evac)
                for w in range(2):
                    use_ss = widx in ss_waves
                    widx += 1
                    cell = {}
                    def valloc(use_ss=use_ss, cell=cell):
                        cell["pv"] = ss_cell() if use_ss else kv_cell()
                    def vstep(c, w=w, cell=cell):
                        for i in range(2):
                            tt = 2 * w + i
                            nc.tensor.matmul(out=cell["pv"][i][:],
                                             lhsT=xts[c][:, 128*tt:128*(tt+1)],
                                             rhs=wv_all[:, CH*c:CH*(c+1)],
                                             start=(c == 0), stop=(c == 7))
                    def vevac(w=w, slab=slab, cell=cell):
                        for i in range(2):
                            g = 4 * slab + 2 * w + i
                            dst = VA[g][:].rearrange("p (h d) -> p h d",
                                                     d=D+1)[:, :, 0:D]
                            src = cell["pv"][i][:].rearrange("p (h d) -> p h d", d=D)
                            nc.vector.tensor_copy(out=dst, in_=src)
                    for c in range(8):
                        if c == 0:
                            steps.append(lambda c=c, a=valloc, v=vstep: (a(), v(c)))
                        else:
                            steps.append(lambda c=c, v=vstep: v(c))
                    steps.append(vevac)
                return steps

            def proj_unit(ti, jc):
                cc, half = ti // 2, ti % 2
                py = kvps.tile([128, 512], F32, name=f"kv{(2*ti+jc) % 2}",
                               tag=f"kv{(2*ti+jc) % 2}")
                for c in range(4):
                    nc.tensor.matmul(out=py[:],
                                     lhsT=OT[c][cc][:, 128*half:128*(half+1)],
                                     rhs=wp_all[:, C*c+512*jc:C*c+512*(jc+1)],
                                     start=(c == 0), stop=(c == 3))
                ysb = msc.tile([128, 512], BF16, name="ysb", tag="ysb", bufs=2)
                nc.vector.tensor_copy(out=ysb[:], in_=py[:])
                nc.sync.dma_start(out=y_d[128*ti:128*(ti+1), 512*jc:512*(jc+1)],
                                  in_=ysb[:])

            def proj_steps(tis):
                steps = []
                for ti in tis:
                    for jc in range(2):
                        steps.append(lambda ti=ti, jc=jc: proj_unit(ti, jc))
                return steps

            # ---- attention j-block ----
            def attn_j(cc, j, tick):
                E = 2 * cc + 2
                o2 = aps.tile([65, 512], F32, name="o2", tag="o2", bufs=2)
                for g in range(E // 2):
                    masked = (g == cc)
                    ss = aps.tile([128, 1024], F32, name="ss", tag="ss", bufs=2)
                    for u in range(2):
                        m = 2 * g + u
                        sl, mm = m // 4, m % 4
                        for h in range(2):
                            nc.tensor.matmul(
                                out=ss[:, 512*h+QC*u:512*h+QC*(u+1)],
                                lhsT=KT[j][sl][64*h:64*(h+1), 128*mm:128*(mm+1)],
                                rhs=QT[j][cc // 2][64*h:64*(h+1), QC*(cc % 2):QC*(cc % 2 + 1)],
                                tile_position=(64 * h, 0),
                                start=True, stop=True)
                    pt = msc.tile([128, 1024], BF16, name="pt", tag="pt", bufs=2)
                    nc.scalar.activation(out=pt[:], in_=ss[:], func=EXP)
                    if masked:
                        nc.vector.tensor_mul(out=pt[:], in0=pt[:],
                                             in1=mk_all[:, cc*1024:(cc+1)*1024])
                    for u in range(2):
                        m = 2 * g + u
                        for h in range(2):
                            nc.tensor.matmul(out=o2[:, QC*h:QC*(h+1)],
                                             lhsT=VA[m][:, 65*(2*j+h):65*(2*j+h)+65],
                                             rhs=pt[:, 512*h+QC*u:512*h+QC*(u+1)],
                                             start=(m == 0 and h == 0),
                                             stop=(m == E - 1),
                                             skip_group_check=True)
                    tick()
                lsb = msc.tile([1, 512], F32, name="lsb", tag="lsb")
                nc.vector.tensor_copy(out=lsb[:], in_=o2[64:65, :])
                rsb = msc.tile([1, 512], F32, name="rsb", tag="rsb")
                nc.vector.reciprocal_approx_fast(rsb[:], lsb[:])
                rbb = msc.tile([64, 512], F32, name="rbb", tag="rbb")
                nc.gpsimd.partition_broadcast(rbb[:], rsb[:])
                for h in range(2):
                    nc.vector.scalar_tensor_tensor(
                        out=OT[j][cc][64*h:64*(h+1), :], in0=o2[0:64, QC*h:QC*(h+1)],
                        scalar=0.0, in1=rbb[:, QC*h:QC*(h+1)],
                        op0=bypass, op1=mult)

            class Pacer:
                def __init__(self, steps, n_g):
                    self.steps, self.n_g = steps, n_g
                    self.fi = 0
                    self.gcount = 0

                def tick(self):
                    self.gcount += 1
                    target = (len(self.steps) * self.gcount) // self.n_g
                    while self.fi < target:
                        self.steps[self.fi]()
                        self.fi += 1

                def flush(self):
                    while self.fi < len(self.steps):
                        self.steps[self.fi]()
                        self.fi += 1

                def prefill(self, n):
                    n = min(n, len(self.steps))
                    while self.fi < n:
                        self.steps[self.fi]()
                        self.fi += 1

            # ---- slab 0 QKV: q-w0, q-w1, k-w0, v-w0 dense (PSUM from the
            # ss ring); k-w1 + v-w1 deferred into chunk 0 as filler.
            s0 = slab_steps(xts0, 0, ss_waves=(0, 1, 2, 3, 4, 5))
            for st in s0[0:27] + s0[36:45]:
                st()
            defer0 = s0[27:36] + s0[45:54]

            # ---- chunks 0-5 as j-interleaved pairs: doubles the span
            # between same-slot o2 allocations so the per-j normalize chain
            # (~3us, DVE+GpSimd) never stalls the PE on the o2 ring. Slab
            # s+1 is paced across its pair.
            for pr in range(3):
                c0, c1 = 2 * pr, 2 * pr + 1
                nxts = load_xt(pr + 1)
                steps = (defer0 if pr == 0 else []) + slab_steps(nxts, pr + 1)
                pacer = Pacer(steps, NJ * ((c0 + 1) + (c1 + 1)))
                pacer.prefill(18 if pr == 0 else 9)
                for j in range(NJ):
                    attn_j(c0, j, pacer.tick)
                    attn_j(c1, j, pacer.tick)
                pacer.flush()

            # ---- chunks 6+7 interleaved j-wise (spreads the exp load);
            # proj fillers ti 0-13 paced across; ti 12/13 (chunk 6's own
            # output rows) land after (6, j=3) completes by construction.
            pacer = Pacer(proj_steps(range(0, 14)), NJ * (7 + 8))
            pacer.prefill(4)
            for j in range(NJ):
                attn_j(6, j, pacer.tick)
                attn_j(7, j, pacer.tick)
            pacer.flush()

            # tail: last two token-tiles of the output projection, c-major
            # so the 12 matmuls not depending on the last normalize issue
            # first (PE is in-order); accumulators borrow the idle ss ring.
            h1 = kv_cell()
            h2 = ss_cell()
            halves = [h1[0], h1[1], h2[0], h2[1]]
            units = [(14, 0), (14, 1), (15, 0), (15, 1)]
            for c in range(4):
                for idx, (ti, jc) in enumerate(units):
                    half = ti % 2
                    nc.tensor.matmul(out=halves[idx][:],
                                     lhsT=OT[c][7][:, 128*half:128*(half+1)],
                                     rhs=wp_all[:, C*c+512*jc:C*c+512*(jc+1)],
                                     start=(c == 0), stop=(c == 3))
            for idx, (ti, jc) in enumerate(units):
                ysb = msc.tile([128, 512], BF16, name="ysb", tag="ysb", bufs=2)
                nc.vector.tensor_copy(out=ysb[:], in_=halves[idx][:])
                nc.sync.dma_start(out=y_d[128*ti:128*(ti+1), 512*jc:512*(jc+1)],
                                  in_=ysb[:])
        pers.release()

    nc.compile()
    return nc


def _get_nc():
    if "nc" not in _cache:
        _cache["nc"] = _build()
    return _cache["nc"]


def _host_prep(x, Wqkv, bqkv, Wproj, bproj):
    bf = ml_dtypes.bfloat16
    x = np.ascontiguousarray(np.asarray(x, dtype=np.float32))
    Wqkv = np.asarray(Wqkv, dtype=np.float32)
    bqkv = np.asarray(bqkv, dtype=np.float32)
    Wproj = np.ascontiguousarray(np.asarray(Wproj, dtype=np.float32))
    bproj = np.asarray(bproj, dtype=np.float32)

    wq = Wqkv[:, :C] * np.float32(0.125)
    wk = Wqkv[:, C:2*C]
    wv = Wqkv[:, 2*C:]
    bq = bqkv[:C] * np.float32(0.125)
    bk = bqkv[C:2*C]
    bv = bqkv[2*C:]
    bpe = (bproj.astype(np.float64) + bv.astype(np.float64) @ Wproj.astype(np.float64)).astype(np.float32)
    _cache["bpe"] = bpe

    pidx = np.arange(128)[:, None]
    fidx = np.arange(QC)[None, :]
    mk = np.zeros((NCH, 128, 4 * QC), dtype=np.float32)
    for cc in range(NCH):
        for u in range(2):
            m = 2 * cc + u
            valid = ((128*m + pidx) <= (QC*cc + fidx)).astype(np.float32)
            for h in range(2):
                mk[cc, :, 512*h+QC*u:512*h+QC*(u+1)] = valid
    mk = mk.astype(bf)

    in_maps = []
    xts = [np.ascontiguousarray(x[b].T).astype(bf) for b in range(B)]
    for core in range(8):
        b, hg = core // 2, core % 2
        cols = slice(CH * hg, CH * (hg + 1))
        in_maps.append(dict(
            xt=xts[b],
            wq=np.ascontiguousarray(wq[:, cols]).astype(bf),
            wk=np.ascontiguousarray(wk[:, cols]).astype(bf),
            wv=np.ascontiguousarray(wv[:, cols]).astype(bf),
            wp=np.ascontiguousarray(Wproj[cols, :]).astype(bf),
            bq=np.ascontiguousarray(bq[cols]).reshape(4, 128, 1),
            bk=np.ascontiguousarray(bk[cols]).reshape(4, 128, 1),
            masks=mk))
    return in_maps


def kernel(x, Wqkv, bqkv, Wproj, bproj):
    nc = _get_nc()
    in_maps = _host_prep(x, Wqkv, bqkv, Wproj, bproj)
    trace = bool(os.environ.get("BASS_TRACE"))
    res = run_bass_kernel_spmd(nc, in_maps, list(range(8)), trace=trace)
    _cache["last_exec_time_ns"] = res.exec_time_ns
    _cache["last_res"] = res
    bpe = _cache["bpe"]
    out = np.empty((B, T, C), dtype=np.float32)
    for b in range(B):
        out[b] = (res.results[2*b]["y"].astype(np.float32)
                  + res.results[2*b+1]["y"].astype(np.float32) + bpe[None, :])
    return out
